# revision 10
# baseline (speedup 1.0000x reference)
"""Fused attention block (QKV proj + per-head RMSNorm + 2D RoPE + softmax
attention + out proj) distributed over 8 TRN2 NeuronCores.

Sharding: core c handles batch c//2 and query-row half c%2 (1024 rows).
K/V are computed for the full 2048 rows on each core (duplicated within a
batch pair) so no cross-core collectives are needed; output rows partition
cleanly across cores.

Row trick: each core's x/positions are rotated so that its query half is
rows 0:1024 — the graph is identical across cores (SPMD), only data
differs. Attention over keys is order-invariant so the rotation is safe.

Softmax runs without max-subtraction: q and k are RMS-normalized so
|q·k|/sqrt(d) <= sqrt(d) = 8 and exp stays comfortably in fp32 range.

RoPE is applied as out = x*C2 + swap(x)*S2 where swap is a negative-step
pairwise read and C2/S2 are per-(tile,half,freq,parity) tables with the
rotation signs and the qk-norm weights folded in.
"""
import sys
sys.path.insert(0, '/opt/trn_rl_repo')
import contextlib
import numpy as np

import concourse.bass as bass
import concourse.tile as tile
from concourse import bacc, mybir
from concourse.bass_utils import run_bass_kernel_spmd
from concourse.masks import make_identity

f32 = mybir.dt.float32
f32r = mybir.dt.float32r
bf16 = mybir.dt.bfloat16
i32 = mybir.dt.int32

B, N, D, H, HD = 4, 2048, 768, 12, 64
NQ = 1024           # query rows per core
NT, NTQ = 16, 8     # 128-row tiles for keys / queries
DT = 6              # 128-row tiles of the embedding dim
PAIRS = 6           # head pairs (2 heads of 64 dims -> 128 partitions)
EPS = 1e-6
TWO_PI = float(2 * np.pi)
HALF_PI = float(np.pi / 2)
SCALE = float(HD ** -0.5)

MM = bf16           # matmul dtype


def build_nc():
    nc = bacc.Bacc("TRN2", target_bir_lowering=False, debug=False, num_devices=8)

    x_d = nc.dram_tensor("x", [N, D], f32, kind="ExternalInput").ap()
    post_d = nc.dram_tensor("pos_t", [128, 2 * NT], i32, kind="ExternalInput").ap()
    wq_d = nc.dram_tensor("Wq", [D, D], f32, kind="ExternalInput").ap()
    wkv_d = nc.dram_tensor("Wkv", [D, 2 * D], f32, kind="ExternalInput").ap()
    wp_d = nc.dram_tensor("Wp", [D, D], f32, kind="ExternalInput").ap()
    bp_d = nc.dram_tensor("bp_bc", [128, D], f32, kind="ExternalInput").ap()
    invf_d = nc.dram_tensor("invf", [128, 16], f32, kind="ExternalInput").ap()
    wcq_d = nc.dram_tensor("wCq", [128, HD], f32, kind="ExternalInput").ap()
    wsq_d = nc.dram_tensor("wSq", [128, HD], f32, kind="ExternalInput").ap()
    wck_d = nc.dram_tensor("wCk", [128, HD], f32, kind="ExternalInput").ap()
    wsk_d = nc.dram_tensor("wSk", [128, HD], f32, kind="ExternalInput").ap()
    ones_d = nc.dram_tensor("ones64", [1, HD], f32, kind="ExternalInput").ap()
    out_d = nc.dram_tensor("out", [NQ, D], f32, kind="ExternalOutput").ap()

    with tile.TileContext(nc) as tc, contextlib.ExitStack() as ctx:
        consts = ctx.enter_context(tc.tile_pool(name="consts", bufs=1))
        persist = ctx.enter_context(tc.tile_pool(name="persist", bufs=1))
        stage = ctx.enter_context(tc.tile_pool(name="stage", bufs=2))
        work = ctx.enter_context(tc.tile_pool(name="work", bufs=2))

        # ---- constants ----
        ident = consts.tile([128, 128], MM)
        make_identity(nc, ident)
        c_zero = consts.tile([128, 1], f32)
        nc.vector.memset(c_zero, 0.0)
        c_eps = consts.tile([128, 1], f32)
        nc.vector.memset(c_eps, EPS)
        ones_r = consts.tile([1, HD], f32r)
        nc.sync.dma_start(out=ones_r, in_=ones_d.bitcast(f32r))
        bp_bc = consts.tile([128, D], f32)
        nc.sync.dma_start(out=bp_bc, in_=bp_d)

        # ---- rope tables: C2/S2 per q and k, [128, NT*64] (t, a, j, e) ----
        C2q = consts.tile([128, 64 * NT], MM)
        S2q = consts.tile([128, 64 * NT], MM)
        C2k = consts.tile([128, 64 * NT], MM)
        S2k = consts.tile([128, 64 * NT], MM)
        with tc.tile_pool(name="tables", bufs=1) as tpool:
            invf_bc = tpool.tile([128, 16], f32)
            nc.sync.dma_start(out=invf_bc, in_=invf_d)
            wpats = {}
            for nm, dram in (("wCq", wcq_d), ("wSq", wsq_d),
                             ("wCk", wck_d), ("wSk", wsk_d)):
                t = tpool.tile([128, HD], f32, name=nm)
                nc.sync.dma_start(out=t, in_=dram)
                wpats[nm] = t
            pos_sb = tpool.tile([128, 2 * NT], i32)
            nc.sync.dma_start(out=pos_sb, in_=post_d)
            posf = tpool.tile([128, 2 * NT], f32)
            nc.vector.tensor_copy(posf, pos_sb)

            ang = tpool.tile([128, 32 * NT], f32)
            for t in range(NT):
                nc.vector.tensor_scalar(
                    out=ang[:, t * 32:t * 32 + 16], in0=invf_bc,
                    scalar1=posf[:, 2 * t:2 * t + 1], scalar2=None,
                    op0=mybir.AluOpType.mult)
                nc.vector.tensor_scalar(
                    out=ang[:, t * 32 + 16:t * 32 + 32], in0=invf_bc,
                    scalar1=posf[:, 2 * t + 1:2 * t + 2], scalar2=None,
                    op0=mybir.AluOpType.mult)
            angc = tpool.tile([128, 32 * NT], f32)
            nc.vector.tensor_scalar(out=angc, in0=ang, scalar1=HALF_PI,
                                    scalar2=None, op0=mybir.AluOpType.add)

            def range_reduce_sin(out, a, tag):
                # out = sin(a - round(a/2pi)*2pi)
                q = tpool.tile([128, 32 * NT], f32, tag="rr_q", name=f"{tag}_q")
                nc.vector.tensor_scalar(out=q, in0=a, scalar1=float(1.0 / TWO_PI),
                                        scalar2=None, op0=mybir.AluOpType.mult)
                qi = tpool.tile([128, 32 * NT], i32, tag="rr_qi", name=f"{tag}_qi")
                nc.vector.tensor_copy(qi, q)
                qf = tpool.tile([128, 32 * NT], f32, tag="rr_qf", name=f"{tag}_qf")
                nc.vector.tensor_copy(qf, qi)
                k = tpool.tile([128, 32 * NT], f32, tag="rr_k", name=f"{tag}_k")
                nc.vector.tensor_scalar(out=k, in0=qf, scalar1=-TWO_PI,
                                        scalar2=None, op0=mybir.AluOpType.mult)
                red = tpool.tile([128, 32 * NT], f32, tag="rr_red", name=f"{tag}_r")
                nc.vector.tensor_add(red, a, k)
                nc.scalar.activation(out, red, mybir.ActivationFunctionType.Sin,
                                     bias=c_zero[:, 0:1])

            sin_all = tpool.tile([128, 32 * NT], f32)
            cos_all = tpool.tile([128, 32 * NT], f32)
            range_reduce_sin(sin_all, ang, "s")
            range_reduce_sin(cos_all, angc, "c")

            # fold signs + norm weights: tab[t,a,j,e] = trig[t,a,j] * w[a,j,e]
            def fold(dst, trig, wpat):
                trig_ap = bass.AP(tensor=trig.tensor, offset=trig.offset,
                                  ap=[trig.ap[0], [32, NT], [16, 2], [1, 16],
                                      [0, 2]])
                w_ap = bass.AP(tensor=wpat.tensor, offset=wpat.offset,
                               ap=[wpat.ap[0], [0, NT], [32, 2], [2, 16], [1, 2]])
                with nc.allow_low_precision("rope tables in matmul dtype"):
                    nc.vector.tensor_tensor(
                        out=dst.rearrange("p (t a j e) -> p t a j e",
                                          t=NT, a=2, j=16),
                        in0=trig_ap, in1=w_ap, op=mybir.AluOpType.mult)

            fold(C2q, cos_all, wpats["wCq"])
            fold(S2q, sin_all, wpats["wSq"])
            fold(C2k, cos_all, wpats["wCk"])
            fold(S2k, sin_all, wpats["wSk"])

        # ---- persistent attention-phase tensors ----
        kT = [persist.tile([128, N], MM, tag=f"kT{p}", name=f"kT{p}")
              for p in range(PAIRS)]
        qT = [persist.tile([128, NQ], MM, tag=f"qT{p}", name=f"qT{p}")
              for p in range(PAIRS)]
        oT = [persist.tile([128, NQ], MM, tag=f"oT{p}", name=f"oT{p}")
              for p in range(PAIRS)]
        v_sb = [persist.tile([128, H * (HD + 1)], MM, tag=f"v{i}", name=f"v{i}")
                for i in range(NT)]
        wp_b = [persist.tile([128, D], MM, tag=f"wp{j}", name=f"wp{j}")
                for j in range(DT)]
        for j in range(DT):
            wf = stage.tile([128, 2 * D], f32, tag="wstage", name=f"wpf{j}")
            nc.sync.dma_start(out=wf[:, 0:D], in_=wp_d[j * 128:(j + 1) * 128, :])
            nc.vector.tensor_copy(wp_b[j], wf[:, 0:D])

        # ---- norm + rope helper ----
        def norm_rope(src_ps, n_i, C2, S2, out_bf, pfx):
            """src_ps: [128, 768] fp32 psum (12 heads x 64). Writes roped MM."""
            kf = work.tile([128, D], f32, tag="kf", bufs=3, name=f"kf{pfx}{n_i}")
            nc.vector.tensor_copy(kf, src_ps)
            sq = work.tile([128, D], f32, tag="sq", bufs=2, name=f"sq{pfx}{n_i}")
            nc.vector.tensor_mul(sq, kf, kf)
            ms = work.tile([128, H], f32, tag="ms", bufs=4, name=f"ms{pfx}{n_i}")
            nc.vector.reduce_sum(ms, sq.rearrange("p (h d) -> p h d", h=H),
                                 axis=mybir.AxisListType.X)
            ln = work.tile([128, H], f32, tag="lnt", bufs=4, name=f"ll{pfx}{n_i}")
            nc.scalar.activation(ln, ms, mybir.ActivationFunctionType.Ln,
                                 scale=float(1.0 / HD), bias=c_eps[:, 0:1])
            rinv = work.tile([128, H], f32, tag="rinv", bufs=4, name=f"rv{pfx}{n_i}")
            nc.scalar.activation(rinv, ln, mybir.ActivationFunctionType.Exp,
                                 scale=-0.5, bias=c_zero[:, 0:1])
            nrm = work.tile([128, D], MM, tag="nrm", bufs=4, name=f"nr{pfx}{n_i}")
            with nc.allow_low_precision("normed qk in matmul dtype"):
                nc.vector.tensor_mul(nrm.rearrange("p (h d) -> p h d", h=H),
                                     kf.rearrange("p (h d) -> p h d", h=H),
                                     rinv.to_broadcast((128, H, HD)))
            # rope: out = nrm*C2[t] + swap(nrm)*S2[t], tables bcast over heads
            def tab(tbl):
                return bass.AP(tensor=tbl.tensor, offset=tbl.offset + n_i * 64,
                               ap=[tbl.ap[0], [0, H], [1, 64]])

            # swap(nrm): pairwise even/odd exchange via negative-step read
            swap = bass.AP(tensor=nrm.tensor, offset=nrm.offset + 1,
                           ap=[nrm.ap[0], [64, H], [2, 32], [-1, 2]])
            m1 = work.tile([128, D], MM, tag="ropem", bufs=6,
                           name=f"m1{pfx}{n_i}")
            with nc.allow_low_precision("rope in matmul dtype"):
                nc.vector.tensor_mul(m1.rearrange("p (h d) -> p h d", h=H),
                                     nrm.rearrange("p (h d) -> p h d", h=H),
                                     tab(C2))
            m2 = work.tile([128, D], MM, tag="ropem", bufs=6,
                           name=f"m2{pfx}{n_i}")
            s2_bc = bass.AP(tensor=S2.tensor, offset=S2.offset + n_i * 64,
                            ap=[S2.ap[0], [0, H], [2, 32], [1, 2]])
            with nc.allow_low_precision("rope in matmul dtype"):
                nc.vector.tensor_mul(
                    m2.rearrange("p (h j e) -> p h j e", h=H, j=32),
                    swap, s2_bc)
            with nc.allow_low_precision("roped qk in matmul dtype"):
                nc.vector.tensor_add(out_bf, m1, m2)

        # ---- proj phase ----
        with tc.tile_pool(name="proj", bufs=1) as proj, \
             tc.tile_pool(name="psum_proj", bufs=2, space="PSUM") as psp:
            wkv_b, wq_b = [], []
            for j in range(DT):
                wf = stage.tile([128, 2 * D], f32, tag="wstage", name=f"wkvf{j}")
                nc.sync.dma_start(out=wf, in_=wkv_d[j * 128:(j + 1) * 128, :])
                wb = proj.tile([128, 2 * D], MM, tag=f"wkv{j}", name=f"wkv{j}")
                nc.vector.tensor_copy(wb, wf)
                wkv_b.append(wb)
            for j in range(DT):
                wf = stage.tile([128, 2 * D], f32, tag="wstage", name=f"wqf{j}")
                nc.sync.dma_start(out=wf[:, 0:D], in_=wq_d[j * 128:(j + 1) * 128, :])
                wb = proj.tile([128, D], MM, tag=f"wq{j}", name=f"wq{j}")
                nc.vector.tensor_copy(wb, wf[:, 0:D])
                wq_b.append(wb)

            for i in range(NT):
                xf = stage.tile([128, D], f32, tag="xstage", name=f"xf{i}")
                nc.sync.dma_start(out=xf, in_=x_d[i * 128:(i + 1) * 128, :])
                xb = stage.tile([128, D], MM, tag="xbf", name=f"xb{i}")
                nc.vector.tensor_copy(xb, xf)
                xtb = []
                for j in range(DT):
                    xt = proj.tile([128, 128], MM, tag="xtb", bufs=8,
                                   name=f"xt{i}_{j}")
                    nc.sync.dma_start(out=xt, in_=xb[:, j * 128:(j + 1) * 128],
                                      transpose=True)
                    xtb.append(xt)
                # kv proj
                kv_ps = psp.tile([128, 2 * D], f32, tag="kv_ps", name=f"kv_ps{i}")
                for j in range(DT):
                    for c in range(3):
                        nc.tensor.matmul(kv_ps[:, c * 512:(c + 1) * 512],
                                         xtb[j],
                                         wkv_b[j][:, c * 512:(c + 1) * 512],
                                         start=(j == 0), stop=(j == DT - 1))
                # v pack: [V_h | 1] per head
                nc.vector.memset(
                    v_sb[i].rearrange("p (h d) -> p h d", h=H)[:, :, HD:HD + 1], 1.0)
                nc.vector.tensor_copy(
                    v_sb[i].rearrange("p (h d) -> p h d", h=H)[:, :, 0:HD],
                    kv_ps[:, D:2 * D].rearrange("p (h d) -> p h d", h=H))
                # k: norm + rope -> MM dtype, transpose per head-pair
                rk = work.tile([128, D], MM, tag="rk", bufs=4, name=f"rk{i}")
                norm_rope(kv_ps[:, 0:D], i, C2k, S2k, rk, "k")
                for p in range(PAIRS):
                    nc.sync.dma_start(out=kT[p][:, i * 128:(i + 1) * 128],
                                      in_=rk[:, p * 128:(p + 1) * 128],
                                      transpose=True)
                # q proj for the first 8 tiles (query rows)
                if i < NTQ:
                    q_ps = psp.tile([128, D], f32, tag="kv_ps", name=f"q_ps{i}")
                    for j in range(DT):
                        nc.tensor.matmul(q_ps[:, 0:512], xtb[j],
                                         wq_b[j][:, 0:512],
                                         start=(j == 0), stop=(j == DT - 1))
                        nc.tensor.matmul(q_ps[:, 512:768], xtb[j],
                                         wq_b[j][:, 512:768],
                                         start=(j == 0), stop=(j == DT - 1))
                    rq = work.tile([128, D], MM, tag="rk", bufs=4, name=f"rq{i}")
                    norm_rope(q_ps, i, C2q, S2q, rq, "q")
                    for p in range(PAIRS):
                        nc.sync.dma_start(out=qT[p][:, i * 128:(i + 1) * 128],
                                          in_=rq[:, p * 128:(p + 1) * 128],
                                          transpose=True)

        # ---- attention per head ----
        with tc.tile_pool(name="attn", bufs=1) as attn, \
             tc.tile_pool(name="psum_attn", bufs=2, space="PSUM") as psa:
            for h in range(H):
                p2, off = h // 2, (h % 2) * HD
                o_ps = psa.tile([HD + 1, NQ], f32, tag="o_ps", name=f"o_ps{h}")
                for kt in range(NT):
                    vk = v_sb[kt][:, h * (HD + 1):(h + 1) * (HD + 1)]
                    for cq in range(2):
                        s_ps = psa.tile([128, 512], f32, tag="s_ps", bufs=4,
                                        name=f"s{h}_{kt}_{cq}")
                        nc.tensor.matmul(s_ps,
                                         kT[p2][off:off + HD,
                                                kt * 128:(kt + 1) * 128],
                                         qT[p2][off:off + HD,
                                                cq * 512:(cq + 1) * 512],
                                         start=True, stop=True)
                        p_b = attn.tile([128, 512], MM, tag="p_b", bufs=6,
                                        name=f"p{h}_{kt}_{cq}")
                        nc.scalar.activation(p_b, s_ps,
                                             mybir.ActivationFunctionType.Exp,
                                             scale=SCALE, bias=c_zero[:, 0:1])
                        nc.tensor.matmul(o_ps[:, cq * 512:(cq + 1) * 512], vk, p_b,
                                         start=(kt == 0), stop=(kt == NT - 1))
                # normalize: oT = o * (1/denom), denom accumulated in row HD
                den_r = attn.tile([1, NQ], f32r, tag="den", bufs=2, name=f"dn{h}")
                nc.scalar.copy(den_r, o_ps[HD:HD + 1, :])
                bc_ps0 = psa.tile([HD, 512], f32, tag="s_ps", bufs=4, name=f"bc{h}_0")
                nc.tensor.matmul(bc_ps0, ones_r, den_r[:, 0:512],
                                 start=True, stop=True)
                bc_ps1 = psa.tile([HD, 512], f32, tag="s_ps", bufs=4, name=f"bc{h}_1")
                nc.tensor.matmul(bc_ps1, ones_r, den_r[:, 512:1024],
                                 start=True, stop=True)
                rec64 = attn.tile([HD, NQ], f32, tag="rec64", bufs=1,
                                  name=f"rc{h}")
                nc.vector.reciprocal_approx_fast(out=rec64[:, 0:512], in_=bc_ps0)
                nc.vector.reciprocal_approx_fast(out=rec64[:, 512:1024], in_=bc_ps1)
                with nc.allow_low_precision("attention output in matmul dtype"):
                    nc.vector.tensor_mul(oT[p2][off:off + HD, :], o_ps[0:HD, :],
                                         rec64)

        # ---- out proj + bias ----
        with tc.tile_pool(name="psum_out", bufs=2, space="PSUM") as pso:
            for i in range(NTQ):
                op_ps = pso.tile([128, D], f32, tag="op_ps", name=f"op{i}")
                for p in range(PAIRS):
                    nc.tensor.matmul(op_ps[:, 0:512],
                                     oT[p][:, i * 128:(i + 1) * 128],
                                     wp_b[p][:, 0:512],
                                     start=(p == 0), stop=(p == PAIRS - 1))
                    nc.tensor.matmul(op_ps[:, 512:768],
                                     oT[p][:, i * 128:(i + 1) * 128],
                                     wp_b[p][:, 512:768],
                                     start=(p == 0), stop=(p == PAIRS - 1))
                o_sb = stage.tile([128, D], f32, tag="o_sb", name=f"osb{i}")
                nc.vector.tensor_add(o_sb, op_ps, bp_bc)
                nc.sync.dma_start(out=out_d[i * 128:(i + 1) * 128, :], in_=o_sb)

    nc.finalize()
    return nc


_NC_CACHE = None


def _get_nc():
    global _NC_CACHE
    if _NC_CACHE is None:
        _NC_CACHE = build_nc()
    return _NC_CACHE


def _wpatterns(w):
    """Build wC/wS [2,16,2] fold patterns from a per-dim weight w[64]."""
    w = np.asarray(w, np.float32)
    wC = np.empty((2, 16, 2), np.float32)
    wS = np.empty((2, 16, 2), np.float32)
    for a in range(2):
        for j in range(16):
            wC[a, j, 0] = w[a * 32 + 2 * j]
            wC[a, j, 1] = w[a * 32 + 2 * j + 1]
            wS[a, j, 0] = -w[a * 32 + 2 * j + 1]
            wS[a, j, 1] = w[a * 32 + 2 * j]
    return (np.tile(wC.reshape(1, 64), (128, 1)),
            np.tile(wS.reshape(1, 64), (128, 1)))


def make_in_maps(x, positions, Wq, Wkv, Wp, bp, qn_w, kn_w, inv_freq):
    x = np.ascontiguousarray(x, dtype=np.float32)
    positions = np.ascontiguousarray(positions, dtype=np.int32)
    wCq, wSq = _wpatterns(qn_w)
    wCk, wSk = _wpatterns(kn_w)
    common = dict(
        Wq=np.ascontiguousarray(Wq, np.float32),
        Wkv=np.ascontiguousarray(Wkv, np.float32),
        Wp=np.ascontiguousarray(Wp, np.float32),
        bp_bc=np.ascontiguousarray(
            np.tile(np.asarray(bp, np.float32).reshape(1, D), (128, 1))),
        invf=np.ascontiguousarray(
            np.tile(np.asarray(inv_freq, np.float32).reshape(1, 16), (128, 1))),
        wCq=wCq, wSq=wSq, wCk=wCk, wSk=wSk,
        ones64=np.ones((1, HD), np.float32),
    )
    in_maps = []
    for c in range(8):
        b, hh = c // 2, c % 2
        xb, pb = x[b], positions[b]
        if hh == 1:
            xb = np.concatenate([xb[NQ:], xb[:NQ]], axis=0)
            pb = np.concatenate([pb[NQ:], pb[:NQ]], axis=0)
        # pos tiled: pos_t[p, 2t+c] = pb[t*128+p, c]
        pos_t = np.ascontiguousarray(
            pb.reshape(NT, 128, 2).transpose(1, 0, 2).reshape(128, 2 * NT))
        in_maps.append(dict(x=np.ascontiguousarray(xb), pos_t=pos_t, **common))
    return in_maps


def kernel(x, positions, Wq, Wkv, Wp, bp, qn_w, kn_w, inv_freq, _trace=False):
    nc = _get_nc()
    in_maps = make_in_maps(x, positions, Wq, Wkv, Wp, bp, qn_w, kn_w, inv_freq)
    res = run_bass_kernel_spmd(nc, in_maps, core_ids=list(range(8)), trace=_trace)
    out = np.empty((B, N, D), np.float32)
    for c in range(8):
        b, hh = c // 2, c % 2
        out[b, hh * NQ:(hh + 1) * NQ] = res.results[c]["out"]
    if _trace:
        kernel.last_exec_time_ns = res.exec_time_ns
        kernel.last_trace = res.instructions_and_trace
    return out


# revision 11
# speedup vs baseline: 1.0392x; 1.0392x over previous
"""Fused attention block (QKV proj + per-head RMSNorm + 2D RoPE + softmax
attention + out proj) distributed over 8 TRN2 NeuronCores.

Sharding: core c handles batch c//2 and query-row half c%2 (1024 rows).
K/V are computed for the full 2048 rows on each core (duplicated within a
batch pair) so no cross-core collectives are needed; output rows partition
cleanly across cores.

Row trick: each core's x/positions are rotated so that its query half is
rows 0:1024 — the graph is identical across cores (SPMD), only data
differs. Attention over keys is order-invariant so the rotation is safe.

Softmax runs without max-subtraction: q and k are RMS-normalized so
|q·k|/sqrt(d) <= sqrt(d) = 8 and exp stays comfortably in fp32 range.

RoPE is applied as out = x*C2 + swap(x)*S2 where swap is a negative-step
pairwise read and C2/S2 are per-(tile,half,freq,parity) tables with the
rotation signs and the qk-norm weights folded in.
"""
import sys
sys.path.insert(0, '/opt/trn_rl_repo')
import contextlib
import numpy as np

import concourse.bass as bass
import concourse.tile as tile
from concourse import bacc, mybir
from concourse.bass_utils import run_bass_kernel_spmd
from concourse.masks import make_identity

f32 = mybir.dt.float32
f32r = mybir.dt.float32r
bf16 = mybir.dt.bfloat16
i32 = mybir.dt.int32

B, N, D, H, HD = 4, 2048, 768, 12, 64
NQ = 1024           # query rows per core
NT, NTQ = 16, 8     # 128-row tiles for keys / queries
DT = 6              # 128-row tiles of the embedding dim
PAIRS = 6           # head pairs (2 heads of 64 dims -> 128 partitions)
EPS = 1e-6
TWO_PI = float(2 * np.pi)
HALF_PI = float(np.pi / 2)
SCALE = float(HD ** -0.5)

MM = bf16           # matmul dtype


def build_nc():
    nc = bacc.Bacc("TRN2", target_bir_lowering=False, debug=False, num_devices=8)

    x_d = nc.dram_tensor("x", [N, D], f32, kind="ExternalInput").ap()
    post_d = nc.dram_tensor("pos_t", [128, 2 * NT], i32, kind="ExternalInput").ap()
    wq_d = nc.dram_tensor("Wq", [D, D], f32, kind="ExternalInput").ap()
    wkv_d = nc.dram_tensor("Wkv", [D, 2 * D], f32, kind="ExternalInput").ap()
    wp_d = nc.dram_tensor("Wp", [D, D], f32, kind="ExternalInput").ap()
    bp_d = nc.dram_tensor("bp_bc", [128, D], f32, kind="ExternalInput").ap()
    invf_d = nc.dram_tensor("invf", [128, 16], f32, kind="ExternalInput").ap()
    wcq_d = nc.dram_tensor("wCq", [128, HD], f32, kind="ExternalInput").ap()
    wsq_d = nc.dram_tensor("wSq", [128, HD], f32, kind="ExternalInput").ap()
    wck_d = nc.dram_tensor("wCk", [128, HD], f32, kind="ExternalInput").ap()
    wsk_d = nc.dram_tensor("wSk", [128, HD], f32, kind="ExternalInput").ap()
    ones_d = nc.dram_tensor("ones64", [1, HD], f32, kind="ExternalInput").ap()
    out_d = nc.dram_tensor("out", [NQ, D], f32, kind="ExternalOutput").ap()

    with tile.TileContext(nc) as tc, contextlib.ExitStack() as ctx:
        consts = ctx.enter_context(tc.tile_pool(name="consts", bufs=1))
        persist = ctx.enter_context(tc.tile_pool(name="persist", bufs=1))
        stage = ctx.enter_context(tc.tile_pool(name="stage", bufs=2))
        work = ctx.enter_context(tc.tile_pool(name="work", bufs=2))

        # ---- constants ----
        ident = consts.tile([128, 128], MM)
        make_identity(nc, ident)
        c_zero = consts.tile([128, 1], f32)
        nc.vector.memset(c_zero, 0.0)
        c_eps = consts.tile([128, 1], f32)
        nc.vector.memset(c_eps, EPS)
        ones_r = consts.tile([1, HD], f32r)
        nc.sync.dma_start(out=ones_r, in_=ones_d.bitcast(f32r))
        bp_bc = consts.tile([128, D], f32)
        nc.sync.dma_start(out=bp_bc, in_=bp_d)

        # ---- rope tables: C2/S2 per q and k, [128, NT*64] (t, a, j, e) ----
        C2q = consts.tile([128, 64 * NT], MM)
        S2q = consts.tile([128, 64 * NT], MM)
        C2k = consts.tile([128, 64 * NT], MM)
        S2k = consts.tile([128, 64 * NT], MM)
        with tc.tile_pool(name="tables", bufs=1) as tpool:
            invf_bc = tpool.tile([128, 16], f32)
            nc.sync.dma_start(out=invf_bc, in_=invf_d)
            wpats = {}
            for nm, dram in (("wCq", wcq_d), ("wSq", wsq_d),
                             ("wCk", wck_d), ("wSk", wsk_d)):
                t = tpool.tile([128, HD], f32, name=nm)
                nc.sync.dma_start(out=t, in_=dram)
                wpats[nm] = t
            pos_sb = tpool.tile([128, 2 * NT], i32)
            nc.sync.dma_start(out=pos_sb, in_=post_d)
            posf = tpool.tile([128, 2 * NT], f32)
            nc.vector.tensor_copy(posf, pos_sb)

            ang = tpool.tile([128, 32 * NT], f32)
            for t in range(NT):
                nc.vector.tensor_scalar(
                    out=ang[:, t * 32:t * 32 + 16], in0=invf_bc,
                    scalar1=posf[:, 2 * t:2 * t + 1], scalar2=None,
                    op0=mybir.AluOpType.mult)
                nc.vector.tensor_scalar(
                    out=ang[:, t * 32 + 16:t * 32 + 32], in0=invf_bc,
                    scalar1=posf[:, 2 * t + 1:2 * t + 2], scalar2=None,
                    op0=mybir.AluOpType.mult)
            angc = tpool.tile([128, 32 * NT], f32)
            nc.vector.tensor_scalar(out=angc, in0=ang, scalar1=HALF_PI,
                                    scalar2=None, op0=mybir.AluOpType.add)

            def range_reduce_sin(out, a, tag):
                # out = sin(a - round(a/2pi)*2pi)
                q = tpool.tile([128, 32 * NT], f32, tag="rr_q", name=f"{tag}_q")
                nc.vector.tensor_scalar(out=q, in0=a, scalar1=float(1.0 / TWO_PI),
                                        scalar2=None, op0=mybir.AluOpType.mult)
                qi = tpool.tile([128, 32 * NT], i32, tag="rr_qi", name=f"{tag}_qi")
                nc.vector.tensor_copy(qi, q)
                qf = tpool.tile([128, 32 * NT], f32, tag="rr_qf", name=f"{tag}_qf")
                nc.vector.tensor_copy(qf, qi)
                k = tpool.tile([128, 32 * NT], f32, tag="rr_k", name=f"{tag}_k")
                nc.vector.tensor_scalar(out=k, in0=qf, scalar1=-TWO_PI,
                                        scalar2=None, op0=mybir.AluOpType.mult)
                red = tpool.tile([128, 32 * NT], f32, tag="rr_red", name=f"{tag}_r")
                nc.vector.tensor_add(red, a, k)
                nc.scalar.activation(out, red, mybir.ActivationFunctionType.Sin,
                                     bias=c_zero[:, 0:1])

            sin_all = tpool.tile([128, 32 * NT], f32)
            cos_all = tpool.tile([128, 32 * NT], f32)
            range_reduce_sin(sin_all, ang, "s")
            range_reduce_sin(cos_all, angc, "c")

            # fold signs + norm weights: tab[t,a,j,e] = trig[t,a,j] * w[a,j,e]
            def fold(dst, trig, wpat):
                trig_ap = bass.AP(tensor=trig.tensor, offset=trig.offset,
                                  ap=[trig.ap[0], [32, NT], [16, 2], [1, 16],
                                      [0, 2]])
                w_ap = bass.AP(tensor=wpat.tensor, offset=wpat.offset,
                               ap=[wpat.ap[0], [0, NT], [32, 2], [2, 16], [1, 2]])
                with nc.allow_low_precision("rope tables in matmul dtype"):
                    nc.vector.tensor_tensor(
                        out=dst.rearrange("p (t a j e) -> p t a j e",
                                          t=NT, a=2, j=16),
                        in0=trig_ap, in1=w_ap, op=mybir.AluOpType.mult)

            fold(C2q, cos_all, wpats["wCq"])
            fold(S2q, sin_all, wpats["wSq"])
            fold(C2k, cos_all, wpats["wCk"])
            fold(S2k, sin_all, wpats["wSk"])

        # ---- persistent attention-phase tensors ----
        kT = [persist.tile([128, N], MM, tag=f"kT{p}", name=f"kT{p}")
              for p in range(PAIRS)]
        qT = [persist.tile([128, NQ], MM, tag=f"qT{p}", name=f"qT{p}")
              for p in range(PAIRS)]
        oT = [persist.tile([128, NQ], MM, tag=f"oT{p}", name=f"oT{p}")
              for p in range(PAIRS)]
        v_sb = [persist.tile([128, H * (HD + 1)], MM, tag=f"v{i}", name=f"v{i}")
                for i in range(NT)]
        wp_b = [persist.tile([128, D], MM, tag=f"wp{j}", name=f"wp{j}")
                for j in range(DT)]
        for j in range(DT):
            wf = stage.tile([128, 2 * D], f32, tag="wstage", name=f"wpf{j}")
            nc.sync.dma_start(out=wf[:, 0:D], in_=wp_d[j * 128:(j + 1) * 128, :])
            nc.scalar.copy(wp_b[j], wf[:, 0:D])

        # ---- norm + rope helper ----
        def norm_rope(src_ps, n_i, C2, S2, out_bf, pfx):
            """src_ps: [128, 768] fp32 psum (12 heads x 64). Writes roped MM."""
            kf = work.tile([128, D], f32, tag="kf", bufs=3, name=f"kf{pfx}{n_i}")
            nc.scalar.copy(kf, src_ps)
            sq = work.tile([128, D], f32, tag="sq", bufs=2, name=f"sq{pfx}{n_i}")
            nc.scalar.activation(sq, kf, mybir.ActivationFunctionType.Square,
                                 bias=c_zero[:, 0:1])
            ms = work.tile([128, H], f32, tag="ms", bufs=4, name=f"ms{pfx}{n_i}")
            nc.vector.reduce_sum(ms, sq.rearrange("p (h d) -> p h d", h=H),
                                 axis=mybir.AxisListType.X)
            ln = work.tile([128, H], f32, tag="lnt", bufs=4, name=f"ll{pfx}{n_i}")
            nc.scalar.activation(ln, ms, mybir.ActivationFunctionType.Ln,
                                 scale=float(1.0 / HD), bias=c_eps[:, 0:1])
            rinv = work.tile([128, H], f32, tag="rinv", bufs=4, name=f"rv{pfx}{n_i}")
            nc.scalar.activation(rinv, ln, mybir.ActivationFunctionType.Exp,
                                 scale=-0.5, bias=c_zero[:, 0:1])
            nrm = work.tile([128, D], MM, tag="nrm", bufs=4, name=f"nr{pfx}{n_i}")
            with nc.allow_low_precision("normed qk in matmul dtype"):
                nc.vector.tensor_mul(nrm.rearrange("p (h d) -> p h d", h=H),
                                     kf.rearrange("p (h d) -> p h d", h=H),
                                     rinv.to_broadcast((128, H, HD)))
            # rope: out = nrm*C2[t] + swap(nrm)*S2[t], tables bcast over heads
            def tab(tbl):
                return bass.AP(tensor=tbl.tensor, offset=tbl.offset + n_i * 64,
                               ap=[tbl.ap[0], [0, H], [1, 64]])

            # swap(nrm): pairwise even/odd exchange via negative-step read
            swap = bass.AP(tensor=nrm.tensor, offset=nrm.offset + 1,
                           ap=[nrm.ap[0], [64, H], [2, 32], [-1, 2]])
            m1 = work.tile([128, D], MM, tag="ropem", bufs=6,
                           name=f"m1{pfx}{n_i}")
            with nc.allow_low_precision("rope in matmul dtype"):
                nc.vector.tensor_mul(m1.rearrange("p (h d) -> p h d", h=H),
                                     nrm.rearrange("p (h d) -> p h d", h=H),
                                     tab(C2))
            m2 = work.tile([128, D], MM, tag="ropem", bufs=6,
                           name=f"m2{pfx}{n_i}")
            s2_bc = bass.AP(tensor=S2.tensor, offset=S2.offset + n_i * 64,
                            ap=[S2.ap[0], [0, H], [2, 32], [1, 2]])
            with nc.allow_low_precision("rope in matmul dtype"):
                nc.vector.tensor_mul(
                    m2.rearrange("p (h j e) -> p h j e", h=H, j=32),
                    swap, s2_bc)
            with nc.allow_low_precision("roped qk in matmul dtype"):
                nc.vector.tensor_add(out_bf, m1, m2)

        # ---- proj phase ----
        with tc.tile_pool(name="proj", bufs=1) as proj, \
             tc.tile_pool(name="psum_proj", bufs=2, space="PSUM") as psp:
            wkv_b, wq_b = [], []
            for j in range(DT):
                wf = stage.tile([128, 2 * D], f32, tag="wstage", name=f"wkvf{j}")
                nc.sync.dma_start(out=wf, in_=wkv_d[j * 128:(j + 1) * 128, :])
                wb = proj.tile([128, 2 * D], MM, tag=f"wkv{j}", name=f"wkv{j}")
                nc.scalar.copy(wb, wf)
                wkv_b.append(wb)
            for j in range(DT):
                wf = stage.tile([128, 2 * D], f32, tag="wstage", name=f"wqf{j}")
                nc.sync.dma_start(out=wf[:, 0:D], in_=wq_d[j * 128:(j + 1) * 128, :])
                wb = proj.tile([128, D], MM, tag=f"wq{j}", name=f"wq{j}")
                nc.scalar.copy(wb, wf[:, 0:D])
                wq_b.append(wb)

            for i in range(NT):
                xf = stage.tile([128, D], f32, tag="xstage", name=f"xf{i}")
                nc.sync.dma_start(out=xf, in_=x_d[i * 128:(i + 1) * 128, :])
                xb = stage.tile([128, D], MM, tag="xbf", name=f"xb{i}")
                nc.scalar.copy(xb, xf)
                xtb = []
                for j in range(DT):
                    tp = psp.tile([128, 128], MM, tag="tp", bufs=2,
                                  name=f"tpx{i}_{j}")
                    nc.tensor.transpose(tp, xb[:, j * 128:(j + 1) * 128], ident)
                    xt = proj.tile([128, 128], MM, tag="xtb", bufs=8,
                                   name=f"xt{i}_{j}")
                    nc.vector.tensor_copy(xt, tp)
                    xtb.append(xt)
                # kv proj
                kv_ps = psp.tile([128, 2 * D], f32, tag="kv_ps", name=f"kv_ps{i}")
                for j in range(DT):
                    for c in range(3):
                        nc.tensor.matmul(kv_ps[:, c * 512:(c + 1) * 512],
                                         xtb[j],
                                         wkv_b[j][:, c * 512:(c + 1) * 512],
                                         start=(j == 0), stop=(j == DT - 1))
                # v pack: [V_h | 1] per head
                nc.vector.memset(
                    v_sb[i].rearrange("p (h d) -> p h d", h=H)[:, :, HD:HD + 1], 1.0)
                nc.vector.tensor_copy(
                    v_sb[i].rearrange("p (h d) -> p h d", h=H)[:, :, 0:HD],
                    kv_ps[:, D:2 * D].rearrange("p (h d) -> p h d", h=H))
                # k: norm + rope -> MM dtype, transpose per head-pair
                rk = work.tile([128, D], MM, tag="rk", bufs=4, name=f"rk{i}")
                norm_rope(kv_ps[:, 0:D], i, C2k, S2k, rk, "k")
                for p in range(PAIRS):
                    tp = psp.tile([128, 128], MM, tag="tp", bufs=2,
                                  name=f"tpk{i}_{p}")
                    nc.tensor.transpose(tp, rk[:, p * 128:(p + 1) * 128], ident)
                    nc.vector.tensor_copy(kT[p][:, i * 128:(i + 1) * 128], tp)
                # q proj for the first 8 tiles (query rows)
                if i < NTQ:
                    q_ps = psp.tile([128, D], f32, tag="kv_ps", name=f"q_ps{i}")
                    for j in range(DT):
                        nc.tensor.matmul(q_ps[:, 0:512], xtb[j],
                                         wq_b[j][:, 0:512],
                                         start=(j == 0), stop=(j == DT - 1))
                        nc.tensor.matmul(q_ps[:, 512:768], xtb[j],
                                         wq_b[j][:, 512:768],
                                         start=(j == 0), stop=(j == DT - 1))
                    rq = work.tile([128, D], MM, tag="rk", bufs=4, name=f"rq{i}")
                    norm_rope(q_ps, i, C2q, S2q, rq, "q")
                    for p in range(PAIRS):
                        tp = psp.tile([128, 128], MM, tag="tp", bufs=2,
                                      name=f"tpq{i}_{p}")
                        nc.tensor.transpose(tp, rq[:, p * 128:(p + 1) * 128], ident)
                        nc.vector.tensor_copy(qT[p][:, i * 128:(i + 1) * 128], tp)

        # ---- attention per head ----
        with tc.tile_pool(name="attn", bufs=1) as attn, \
             tc.tile_pool(name="psum_attn", bufs=2, space="PSUM") as psa:
            for h in range(H):
                p2, off = h // 2, (h % 2) * HD
                o_ps = psa.tile([HD + 1, NQ], f32, tag="o_ps", name=f"o_ps{h}")
                for kt in range(NT):
                    vk = v_sb[kt][:, h * (HD + 1):(h + 1) * (HD + 1)]
                    for cq in range(2):
                        s_ps = psa.tile([128, 512], f32, tag="s_ps", bufs=4,
                                        name=f"s{h}_{kt}_{cq}")
                        nc.tensor.matmul(s_ps,
                                         kT[p2][off:off + HD,
                                                kt * 128:(kt + 1) * 128],
                                         qT[p2][off:off + HD,
                                                cq * 512:(cq + 1) * 512],
                                         start=True, stop=True)
                        p_b = attn.tile([128, 512], MM, tag="p_b", bufs=6,
                                        name=f"p{h}_{kt}_{cq}")
                        nc.scalar.activation(p_b, s_ps,
                                             mybir.ActivationFunctionType.Exp,
                                             scale=SCALE, bias=c_zero[:, 0:1])
                        nc.tensor.matmul(o_ps[:, cq * 512:(cq + 1) * 512], vk, p_b,
                                         start=(kt == 0), stop=(kt == NT - 1))
                # normalize: oT = o * (1/denom), denom accumulated in row HD
                den_r = attn.tile([1, NQ], f32r, tag="den", bufs=2, name=f"dn{h}")
                nc.scalar.copy(den_r, o_ps[HD:HD + 1, :])
                bc_ps0 = psa.tile([HD, 512], f32, tag="s_ps", bufs=4, name=f"bc{h}_0")
                nc.tensor.matmul(bc_ps0, ones_r, den_r[:, 0:512],
                                 start=True, stop=True)
                bc_ps1 = psa.tile([HD, 512], f32, tag="s_ps", bufs=4, name=f"bc{h}_1")
                nc.tensor.matmul(bc_ps1, ones_r, den_r[:, 512:1024],
                                 start=True, stop=True)
                rec64 = attn.tile([HD, NQ], f32, tag="rec64", bufs=1,
                                  name=f"rc{h}")
                nc.vector.reciprocal_approx_fast(out=rec64[:, 0:512], in_=bc_ps0)
                nc.vector.reciprocal_approx_fast(out=rec64[:, 512:1024], in_=bc_ps1)
                with nc.allow_low_precision("attention output in matmul dtype"):
                    nc.vector.tensor_mul(oT[p2][off:off + HD, :], o_ps[0:HD, :],
                                         rec64)

        # ---- out proj + bias ----
        with tc.tile_pool(name="psum_out", bufs=2, space="PSUM") as pso:
            for i in range(NTQ):
                op_ps = pso.tile([128, D], f32, tag="op_ps", name=f"op{i}")
                for p in range(PAIRS):
                    nc.tensor.matmul(op_ps[:, 0:512],
                                     oT[p][:, i * 128:(i + 1) * 128],
                                     wp_b[p][:, 0:512],
                                     start=(p == 0), stop=(p == PAIRS - 1))
                    nc.tensor.matmul(op_ps[:, 512:768],
                                     oT[p][:, i * 128:(i + 1) * 128],
                                     wp_b[p][:, 512:768],
                                     start=(p == 0), stop=(p == PAIRS - 1))
                o_sb = stage.tile([128, D], f32, tag="o_sb", name=f"osb{i}")
                nc.vector.tensor_add(o_sb, op_ps, bp_bc)
                nc.sync.dma_start(out=out_d[i * 128:(i + 1) * 128, :], in_=o_sb)

    nc.finalize()
    return nc


_NC_CACHE = None


def _get_nc():
    global _NC_CACHE
    if _NC_CACHE is None:
        _NC_CACHE = build_nc()
    return _NC_CACHE


def _wpatterns(w):
    """Build wC/wS [2,16,2] fold patterns from a per-dim weight w[64]."""
    w = np.asarray(w, np.float32)
    wC = np.empty((2, 16, 2), np.float32)
    wS = np.empty((2, 16, 2), np.float32)
    for a in range(2):
        for j in range(16):
            wC[a, j, 0] = w[a * 32 + 2 * j]
            wC[a, j, 1] = w[a * 32 + 2 * j + 1]
            wS[a, j, 0] = -w[a * 32 + 2 * j + 1]
            wS[a, j, 1] = w[a * 32 + 2 * j]
    return (np.tile(wC.reshape(1, 64), (128, 1)),
            np.tile(wS.reshape(1, 64), (128, 1)))


def make_in_maps(x, positions, Wq, Wkv, Wp, bp, qn_w, kn_w, inv_freq):
    x = np.ascontiguousarray(x, dtype=np.float32)
    positions = np.ascontiguousarray(positions, dtype=np.int32)
    wCq, wSq = _wpatterns(qn_w)
    wCk, wSk = _wpatterns(kn_w)
    common = dict(
        Wq=np.ascontiguousarray(Wq, np.float32),
        Wkv=np.ascontiguousarray(Wkv, np.float32),
        Wp=np.ascontiguousarray(Wp, np.float32),
        bp_bc=np.ascontiguousarray(
            np.tile(np.asarray(bp, np.float32).reshape(1, D), (128, 1))),
        invf=np.ascontiguousarray(
            np.tile(np.asarray(inv_freq, np.float32).reshape(1, 16), (128, 1))),
        wCq=wCq, wSq=wSq, wCk=wCk, wSk=wSk,
        ones64=np.ones((1, HD), np.float32),
    )
    in_maps = []
    for c in range(8):
        b, hh = c // 2, c % 2
        xb, pb = x[b], positions[b]
        if hh == 1:
            xb = np.concatenate([xb[NQ:], xb[:NQ]], axis=0)
            pb = np.concatenate([pb[NQ:], pb[:NQ]], axis=0)
        # pos tiled: pos_t[p, 2t+c] = pb[t*128+p, c]
        pos_t = np.ascontiguousarray(
            pb.reshape(NT, 128, 2).transpose(1, 0, 2).reshape(128, 2 * NT))
        in_maps.append(dict(x=np.ascontiguousarray(xb), pos_t=pos_t, **common))
    return in_maps


def kernel(x, positions, Wq, Wkv, Wp, bp, qn_w, kn_w, inv_freq, _trace=False):
    nc = _get_nc()
    in_maps = make_in_maps(x, positions, Wq, Wkv, Wp, bp, qn_w, kn_w, inv_freq)
    res = run_bass_kernel_spmd(nc, in_maps, core_ids=list(range(8)), trace=_trace)
    out = np.empty((B, N, D), np.float32)
    for c in range(8):
        b, hh = c // 2, c % 2
        out[b, hh * NQ:(hh + 1) * NQ] = res.results[c]["out"]
    if _trace:
        kernel.last_exec_time_ns = res.exec_time_ns
        kernel.last_trace = res.instructions_and_trace
    return out
